# revision 1
# baseline (speedup 1.0000x reference)
"""Trainium2 Bass kernel for nn_FWQ adaptive featurewise quantization.

Contract: kernel(**inputs) takes the FULL inputs (x: [8, 2048, 1024] f32),
distributes across 8 NeuronCores internally, and returns the full output in
the same structure as the reference: (out, 0.0, 0.0).

Plan (data-parallel across rows, per the sharding hint):
  K1 (device): per-core per-column min/max partials + column sums.
  host: combine stats (all-reduce), run the faithful f32 scalar chain
        (argsort by range, 8-candidate water-filling bisections, argmin,
        quantizer setup) exactly mirroring the reference's jax-on-CPU
        semantics, producing per-column affine params.
  K2 (device): per-core elementwise quantization
        y = min(rne_i32(relu(alpha*x + bias)) * beta + gamma, ybound)
        via PE transpose -> ACT(Relu, scale, bias)->i32 -> ACT(Identity,
        beta, gamma) -> PE transpose back -> DVE min clamp -> DMA out.
"""
import numpy as np

import orjson
import concourse.bass as bass
import concourse.tile as tile
from concourse import mybir
from concourse import bass_utils

dt = mybir.dt
AF = mybir.ActivationFunctionType
ALU = mybir.AluOpType

N_CORES = 8
P = 128
R = 2048          # rows per core  (8*2048*1024 / 8 cores / 1024 cols)
D = 1024          # columns
NT = R // P       # 16 row-tiles per core
NJ = D // P       # 8 column blocks

F32 = np.float32
Q_EP = F32(200.0)
CAVA = F32(62.0)
TOL = F32(1e-4)
MAX_ITER = 100
LN2 = float(np.log(2.0))

# ---------------------------------------------------------------------------
# Workaround: this toolchain accepts at most ONE sync wait per instruction.
# Split every multi-wait instruction into standalone single-wait
# EventSemaphore instructions placed immediately before it.
# ---------------------------------------------------------------------------
_swx_counter = [0]


def _split_multiwait_bir(bir_bytes: bytes) -> bytes:
    d = orjson.loads(bir_bytes)
    changed = False
    for fn in d.get("functions") or []:
        for bb in fn.get("basic_blocks") or fn.get("blocks") or []:
            insts = bb.get("instructions") or []
            new_insts = []
            for inst in insts:
                si = inst.get("sync_info")
                waits = (si or {}).get("on_wait") or []
                if len(waits) > 1:
                    changed = True
                    for w in waits[:-1]:
                        _swx_counter[0] += 1
                        new_insts.append(
                            {
                                "engine": inst["engine"],
                                "ins": [],
                                "outs": [],
                                "name": f"SWX-{_swx_counter[0]}",
                                "opcode": "EventSemaphore",
                                "sync_info": {"on_update": [], "on_wait": [w]},
                                "debug": inst.get("debug", 0),
                            }
                        )
                    si["on_wait"] = [waits[-1]]
                new_insts.append(inst)
            bb["instructions"] = new_insts
    return orjson.dumps(d) if changed else bir_bytes


_patched = [False]


def _install_patch():
    if _patched[0]:
        return
    _patched[0] = True
    orig = bass.Bass.to_json_bytes

    def patched(self, *a, **k):
        return _split_multiwait_bir(orig(self, *a, **k))

    bass.Bass.to_json_bytes = patched


# ---------------------------------------------------------------------------
# Device kernels
# ---------------------------------------------------------------------------

def _build_k1(reps=1):
    """Stats: x [R, D] -> pmin/pmax [P, D] partials + sums [1, D]."""
    nc = bass.Bass(target_bir_lowering=False, debug=True)
    x = nc.dram_tensor("x", [R, D], dt.float32, kind="ExternalInput")
    pmin_d = nc.dram_tensor("pmin", [P, D], dt.float32, kind="ExternalOutput")
    pmax_d = nc.dram_tensor("pmax", [P, D], dt.float32, kind="ExternalOutput")
    sums_d = nc.dram_tensor("sums", [1, D], dt.float32, kind="ExternalOutput")

    xv = x.ap().rearrange("(t p) d -> t p d", p=P)

    with tile.TileContext(nc) as tc:
        with (
            tc.tile_pool(name="big", bufs=1) as bigp,
            tc.tile_pool(name="work", bufs=2) as work,
            tc.tile_pool(name="psum", bufs=1, space="PSUM") as psp,
        ):
            ones = bigp.tile([P, 1], dt.float32)
            nc.vector.memset(ones[:], 1.0)
            for _ in range(reps):
                big = bigp.tile([P, NT, D], dt.float32, tag="big")
                for t in range(NT):
                    nc.sync.dma_start(out=big[:, t, :], in_=xv[t])

                bv = big[:].rearrange("p t d -> p d t")
                pmax = work.tile([P, D], dt.float32, tag="pmax")
                nc.vector.tensor_reduce(out=pmax[:], in_=bv,
                                        axis=mybir.AxisListType.X, op=ALU.max)
                pmin = work.tile([P, D], dt.float32, tag="pmin")
                nc.vector.tensor_reduce(out=pmin[:], in_=bv,
                                        axis=mybir.AxisListType.X, op=ALU.min)
                nc.sync.dma_start(out=pmin_d.ap(), in_=pmin[:])
                nc.sync.dma_start(out=pmax_d.ap(), in_=pmax[:])

                psum_s = psp.tile([1, D], dt.float32, tag="ps")
                for j in range(2):
                    for t in range(NT):
                        nc.tensor.matmul(
                            psum_s[0:1, j * 512 : (j + 1) * 512],
                            ones[:, 0:1],
                            big[:, t, j * 512 : (j + 1) * 512],
                            start=(t == 0),
                            stop=(t == NT - 1),
                        )
                ssum = work.tile([1, D], dt.float32, tag="ssum")
                nc.vector.tensor_copy(ssum[:], psum_s[:])
                nc.sync.dma_start(out=sums_d.ap(), in_=ssum[0:1, :])
    return nc


def _build_k2(reps=1):
    """Quantize: y = min(rne_i32(relu(al*x+bi))*be+ga, ybound) per column."""
    nc = bass.Bass(target_bir_lowering=False, debug=True)
    x = nc.dram_tensor("x", [R, D], dt.float32, kind="ExternalInput")
    alpha = nc.dram_tensor("alpha", [D], dt.float32, kind="ExternalInput")
    bias_ = nc.dram_tensor("bias_", [D], dt.float32, kind="ExternalInput")
    beta = nc.dram_tensor("beta", [D], dt.float32, kind="ExternalInput")
    gamma = nc.dram_tensor("gamma", [D], dt.float32, kind="ExternalInput")
    ybound = nc.dram_tensor("ybound", [D], dt.float32, kind="ExternalInput")
    ident = nc.dram_tensor("ident", [P, P], dt.float32, kind="ExternalInput")
    y = nc.dram_tensor("y", [R, D], dt.float32, kind="ExternalOutput")

    NU = 4
    TPU = NT // NU

    xv = x.ap().rearrange("(u t p) d -> u t p d", t=TPU, p=P)
    yv = y.ap().rearrange("(u t p) d -> u t p d", t=TPU, p=P)

    with tile.TileContext(nc) as tc:
        with (
            tc.tile_pool(name="singles", bufs=1) as singles,
            tc.tile_pool(name="xin", bufs=2) as xpool,
            tc.tile_pool(name="ipool", bufs=3) as ipool,
            tc.tile_pool(name="ypool", bufs=2) as ypool,
            tc.tile_pool(name="opool", bufs=3) as opool,
            tc.tile_pool(name="pt", bufs=2, space="PSUM") as ptp,
            tc.tile_pool(name="po", bufs=2, space="PSUM") as pop,
        ):
            def load_pm(name, dram):
                t = singles.tile([P, NJ], dt.float32, tag=name)
                nc.sync.dma_start(
                    out=t[:], in_=dram.ap().rearrange("(j p) -> p j", p=P)
                )
                return t

            al_t = load_pm("al", alpha)
            bi_t = load_pm("bi", bias_)
            be_t = load_pm("be", beta)
            ga_t = load_pm("ga", gamma)

            yb_t = singles.tile([P, D], dt.float32, tag="yb")
            yb_ap = bass.AP(tensor=ybound.ap().tensor, offset=ybound.ap().offset,
                            ap=[[0, P], [1, D]])
            nc.sync.dma_start(out=yb_t[:], in_=yb_ap)

            id_t = singles.tile([P, P], dt.float32, tag="id")
            nc.sync.dma_start(out=id_t[:], in_=ident.ap())

            for _ in range(reps):
                for u in range(NU):
                    xin = xpool.tile([P, TPU, D], dt.float32, tag="xin")
                    for t in range(TPU):
                        nc.sync.dma_start(out=xin[:, t, :], in_=xv[u, t])

                    yts = []
                    for j in range(NJ):
                        pt = ptp.tile([P, TPU * P], dt.float32, tag="pt")
                        for t in range(TPU):
                            nc.tensor.matmul(
                                pt[:, t * P : (t + 1) * P],
                                xin[:, t, j * P : (j + 1) * P],
                                id_t[:],
                                is_transpose=True,
                                start=(t == 0),
                                stop=(t == TPU - 1),
                            )
                        it = ipool.tile([P, TPU * P], dt.int32, tag="it")
                        nc.scalar.activation(out=it[:], in_=pt[:], func=AF.Relu,
                                             scale=al_t[:, j : j + 1],
                                             bias=bi_t[:, j : j + 1])
                        yt = ypool.tile([P, TPU * P], dt.float32, tag=f"yt{j}")
                        nc.scalar.activation(out=yt[:], in_=it[:],
                                             func=AF.Identity,
                                             scale=be_t[:, j : j + 1],
                                             bias=ga_t[:, j : j + 1])
                        yts.append(yt)

                    for t in range(TPU):
                        po = pop.tile([P, D], dt.float32, tag="po")
                        for j in range(NJ):
                            nc.tensor.matmul(
                                po[:, j * P : (j + 1) * P],
                                yts[j][:, t * P : (t + 1) * P],
                                id_t[:],
                                is_transpose=True,
                                start=(j % 4 == 0),
                                stop=(j % 4 == 3),
                            )
                        ot = opool.tile([P, D], dt.float32, tag="ot")
                        nc.vector.tensor_tensor(out=ot[:], in0=po[:], in1=yb_t[:],
                                                op=ALU.min)
                        nc.sync.dma_start(out=yv[u, t], in_=ot[:])
    return nc


# ---------------------------------------------------------------------------
# Host-side faithful scalar chain (mirrors reference.py on CPU in f32)
# ---------------------------------------------------------------------------

def _compute_Q(u):
    with np.errstate(all="ignore"):
        v = (u * np.sqrt(F32(81.0) - F32(12.0) * u) + F32(9.0) * u) ** F32(1.0 / 3.0)
        Q = (
            F32((2.0 / 3.0) ** (1.0 / 3.0)) * (u / v)
            + v / F32(2.0 ** (1.0 / 3.0) * 3.0 ** (2.0 / 3.0))
            + F32(1.0)
        )
        return np.clip(Q, F32(2.0), F32(2.0**32))


def _solve_levels(r_s, mask, a0, Bf):
    r2ln2 = r_s.astype(F32) ** 2 * F32(LN2)
    u0c = a0 * a0 * Bf * F32(LN2)

    def u_of(nu):
        with np.errstate(all="ignore"):
            u_cols = r2ln2 / (F32(2.0) * nu)
            u0 = u0c / nu
        return np.concatenate([[u0], u_cols]).astype(F32)

    def bit_sum(nu):
        with np.errstate(all="ignore"):
            lg = np.log2(_compute_Q(u_of(nu)))
            return F32(lg[0] + np.sum(np.where(mask, lg[1:], F32(0.0)), dtype=F32))

    lo, hi, numid = F32(1e-12), F32(1e6), F32(0.0)
    it = 0
    done = False
    while it < MAX_ITER and not done:
        mid = F32((lo + hi) * F32(0.5))
        bs = bit_sum(mid)
        if bool(bs > CAVA):   # NaN -> False, like the jax while_loop
            lo = mid
        else:
            hi = mid
        numid = mid
        it += 1
        done = bool(np.abs(bs - CAVA) < TOL)
    return _compute_Q(u_of(numid))


def _afq_params(colmin, colmax, colsum, N):
    colmin = colmin.astype(F32)
    colmax = colmax.astype(F32)
    colsum = colsum.astype(F32)
    Dn = colmin.shape[0]
    Bf = F32(N)

    ranges = colmax - colmin
    idx_sorted = np.argsort(-ranges, kind="stable")
    r_s = ranges[idx_sorted]
    means = colsum / F32(N)
    a0 = F32(means.max() - means.min())

    candidates = np.unique(np.linspace(1, Dn // 2, num=8, dtype=int))
    col = np.arange(Dn)
    errs = []
    Qs = []
    with np.errstate(all="ignore"):
        for M in candidates:
            mask = col < int(M)
            Q_all = _solve_levels(r_s, mask, a0, Bf)
            Qe = Q_all[1:]
            err_two = np.sum(
                np.where(mask, r_s**2 * Bf / (F32(4.0) * (Qe - F32(1.0)) ** 2),
                         F32(0.0)), dtype=F32)
            err_m1 = np.sum(
                np.where(mask, F32(0.0), r_s**2 * Bf / F32(2.0)), dtype=F32)
            err_m2 = F32(
                a0 * a0 * Bf * F32(float(Dn - int(M)))
                / (F32(2.0) * (Q_all[0] - F32(1.0)) ** 2))
            errs.append(F32(err_two + err_m1 + err_m2))
            Qs.append(Q_all)
    errs = np.asarray(errs, F32)
    best = int(np.argmin(errs))  # NaN-first, matching numpy/jax-on-CPU
    M_star = int(candidates[best])
    Q_list = Qs[best]

    two_mask_sorted = col < M_star
    inf = F32(np.inf)

    a_min = colmin[idx_sorted]
    a_max = colmax[idx_sorted]
    with np.errstate(all="ignore"):
        min_low = F32(np.where(two_mask_sorted, a_min, inf).min())
        min_up = F32(np.where(two_mask_sorted, a_min, -inf).max())
        max_low = F32(np.where(two_mask_sorted, a_max, inf).min())
        max_up = F32(np.where(two_mask_sorted, a_max, -inf).max())

        def _uq_idx(xx, lo, hi, Q):
            xc = np.clip(xx, lo, hi)
            return np.round((xc - lo) / (hi - lo) * (Q - F32(1.0)))

        min_q = (
            _uq_idx(a_min, min_low, min_up, Q_EP) / (Q_EP - F32(1.0))
            * (min_up - min_low) + min_low
        ).astype(F32)
        max_q = (
            _uq_idx(a_max, max_low, max_up, Q_EP) / (Q_EP - F32(1.0))
            * (max_up - max_low) + max_low
        ).astype(F32)
        Q_entry = np.round(Q_list[1:]).astype(F32)

        mmask = ~two_mask_sorted
        a_mean = means[idx_sorted]
        qmin = F32(np.where(mmask, a_mean, inf).min())
        qmax = F32(np.where(mmask, a_mean, -inf).max())
        Q0 = F32(np.round(Q_list[0]))
        mean_q = (
            _uq_idx(a_mean, qmin, qmax, Q0) / Q0 * (qmax - qmin) + qmin
        ).astype(F32)

        alpha_s = np.empty(Dn, F32)
        bias_s = np.empty(Dn, F32)
        beta_s = np.empty(Dn, F32)
        gamma_s = np.empty(Dn, F32)
        ybound_s = np.empty(Dn, F32)

        ts = two_mask_sorted
        alpha_s[ts] = (Q_entry[ts] - F32(1.0)) / (max_q[ts] - min_q[ts])
        bias_s[ts] = -(min_q[ts] * alpha_s[ts])
        beta_s[ts] = (max_q[ts] - min_q[ts]) / Q_entry[ts]
        gamma_s[ts] = min_q[ts]
        ybound_s[ts] = (
            (Q_entry[ts].astype(np.float64) - 1.0) * beta_s[ts].astype(np.float64)
            + gamma_s[ts].astype(np.float64)
        ).astype(F32)
        ms = mmask
        alpha_s[ms] = F32(0.0)
        bias_s[ms] = F32(0.0)
        beta_s[ms] = F32(0.0)
        gamma_s[ms] = mean_q[ms]
        ybound_s[ms] = mean_q[ms]

    inv = np.empty(Dn, np.int64)
    inv[idx_sorted] = np.arange(Dn)
    return {
        "alpha": np.ascontiguousarray(alpha_s[inv]),
        "bias": np.ascontiguousarray(bias_s[inv]),
        "beta": np.ascontiguousarray(beta_s[inv]),
        "gamma": np.ascontiguousarray(gamma_s[inv]),
        "ybound": np.ascontiguousarray(ybound_s[inv]),
    }


# ---------------------------------------------------------------------------
# Entry point
# ---------------------------------------------------------------------------

_cache = {}


def _get_nc(which, reps=1):
    key = (which, reps)
    if key not in _cache:
        _install_patch()
        _cache[key] = _build_k1(reps) if which == "k1" else _build_k2(reps)
    return _cache[key]


def kernel(x):
    x = np.asarray(x)
    assert x.dtype == np.float32
    x_shape = x.shape
    xf = np.ascontiguousarray(x.reshape(-1, x_shape[-1]))
    N = xf.shape[0]
    shards = [xf[c * R : (c + 1) * R] for c in range(N_CORES)]

    # K1: device column stats
    nc1 = _get_nc("k1")
    res1 = bass_utils.run_bass_kernel_spmd(
        nc1, [{"x": s} for s in shards], core_ids=list(range(N_CORES))
    ).results
    colmin = np.min(np.stack([r["pmin"] for r in res1]).reshape(-1, D), axis=0)
    colmax = np.max(np.stack([r["pmax"] for r in res1]).reshape(-1, D), axis=0)
    colsum = np.zeros(D, F32)
    for r in res1:
        colsum = colsum + r["sums"][0]

    # host scalar chain
    p = _afq_params(colmin, colmax, colsum, N)
    ident = np.eye(P, dtype=np.float32)

    # K2: device quantization
    nc2 = _get_nc("k2")
    in_maps = [
        dict(x=s, alpha=p["alpha"], bias_=p["bias"], beta=p["beta"],
             gamma=p["gamma"], ybound=p["ybound"], ident=ident)
        for s in shards
    ]
    res2 = bass_utils.run_bass_kernel_spmd(
        nc2, in_maps, core_ids=list(range(N_CORES))
    ).results
    out = np.concatenate([r["y"] for r in res2], axis=0)
    return out.reshape(x_shape), 0.0, 0.0
